# revision 31
# baseline (speedup 1.0000x reference)
"""Two-layer GAT on 8 Trainium2 NeuronCores.

Sharding: destination-node partitioning (1250 dst nodes per core, padded to
1280).  Each core computes the dense feature matmul for its own node chunk,
feature+logit tables are AllGathered, and each core processes the edges whose
destination lands in its chunk: indexed row gathers (dma_gather, spread over
4 SWDGE queues) for source features, edge softmax without max-subtraction,
and aggregation as one-hot scatter-matmuls on the tensor engine.

Layer-1 features are stored d-major (f = d*H0 + h) so the per-edge alpha
weighting runs in the DVE 2x packed mode (innermost stride-1 dim = heads).
The attention-projection columns (Wl/Wr) are precomputed on the host.
"""
import numpy as np
import ml_dtypes

N_NODES = 10000
N_EDGES = 320000
IN_DIM = 512
HID = 64
H0 = 8
OUT_D = 64
NEG = 0.2
NCORES = 8
NPER = 1250          # real nodes per core
LOC = 1280           # padded rows per core
FULL = LOC * NCORES  # 10240 padded-global rows
NCHUNK = 10          # dst chunks (of 128) per core
NQ = 4               # gather sub-calls per chunk == SWDGE queues

_cache = {}


def _build(nslab):
    import concourse.bacc as bacc
    import concourse.mybir as mybir
    import concourse.tile as tile

    f32 = mybir.dt.float32
    bf16 = mybir.dt.bfloat16
    u16 = mybir.dt.uint16
    u8 = mybir.dt.uint8
    f8 = mybir.dt.float8e4
    i16 = mybir.dt.int16
    Alu = mybir.AluOpType
    Act = mybir.ActivationFunctionType

    # sub-gather sizes: ceil-split so nslab need not divide by NQ
    nqs = [nslab // NQ + (1 if i < nslab % NQ else 0) for i in range(NQ)]
    offs = [0]
    for s_ in nqs:
        offs.append(offs[-1] + s_)
    EPAD = NCHUNK * nslab * 128
    NI = nslab * 128            # gather idxs per chunk

    nc = bacc.Bacc("TRN2", target_bir_lowering=False, debug=False,
                   enable_asserts=True, num_devices=NCORES,
                   num_swdge_queues=4)

    # ---------------- I/O ----------------
    hT_d = nc.dram_tensor("hT", [IN_DIM, LOC], bf16, kind="ExternalInput")
    W1_d = nc.dram_tensor("W1", [IN_DIM, IN_DIM], bf16, kind="ExternalInput")
    Wl1_d = nc.dram_tensor("Wl1", [IN_DIM, H0], bf16, kind="ExternalInput")
    Wr1_d = nc.dram_tensor("Wr1", [IN_DIM, H0], bf16, kind="ExternalInput")
    W2c_d = nc.dram_tensor("W2c", [IN_DIM, OUT_D + 2], bf16,
                           kind="ExternalInput")
    srcI_d = nc.dram_tensor("srcI", [128, EPAD // 16], i16, kind="ExternalInput")
    dOffB_d = nc.dram_tensor("dOffB", [128, EPAD], f8, kind="ExternalInput")
    stOH_d = nc.dram_tensor("stOH", [128, EPAD], f8, kind="ExternalInput")
    id_d = nc.dram_tensor("ident", [128, 128], f32, kind="ExternalInput")
    y_d = nc.dram_tensor("y", [LOC, OUT_D], f32, kind="ExternalOutput")

    # ---------------- internal DRAM ----------------
    fx1_loc = nc.dram_tensor("fx1_loc", [LOC, 768], u8)
    fx1_full = nc.dram_tensor("fx1_full", [FULL, 768], u8, addr_space="Shared")
    fx2_loc = nc.dram_tensor("fx2_loc", [LOC, 128], u16)
    fx2_full = nc.dram_tensor("fx2_full", [FULL, 128], u16, addr_space="Shared")

    K1 = IN_DIM // 128  # 4

    with tile.TileContext(nc) as tc:
        with (
            tc.tile_pool(name="const", bufs=1) as cp,
            tc.tile_pool(name="work", bufs=2) as wp,
            tc.tile_pool(name="xt", bufs=1) as xp,
        ):
            def load_const(name, dram, shape, dtype):
                t = cp.tile(shape, dtype, tag=name)
                nc.sync.dma_start(t[:], dram)
                return t

            id_t = load_const("ident", id_d[:], [128, 128], f32)
            srcI_t = load_const("srcI", srcI_d[:], [128, EPAD // 16], i16)
            W1_t = [load_const(f"W1_{s}", W1_d[s * 128:(s + 1) * 128, :],
                               [128, IN_DIM], bf16) for s in range(K1)]
            Wl1_t = [load_const(f"Wl1_{s}", Wl1_d[s * 128:(s + 1) * 128, :],
                                [128, H0], bf16) for s in range(K1)]
            Wr1_t = [load_const(f"Wr1_{s}", Wr1_d[s * 128:(s + 1) * 128, :],
                                [128, H0], bf16) for s in range(K1)]
            W2c_t = [load_const(f"W2c_{s}", W2c_d[s * 128:(s + 1) * 128, :],
                                [128, OUT_D + 2], bf16) for s in range(K1)]
            # layer-2 stationary operand (x transposed), filled in phase C
            xT_t = [xp.tile([128, LOC], bf16, tag=f"xT_{s}", name=f"xT_{s}")
                    for s in range(K1)]
            er1_sb, er2_sb = [], []

            # ---- phase B: layer-1 features + logit tables ----
            hw_pool = tc.tile_pool(name="hw", bufs=1)
            hp = hw_pool.__enter__()
            hT_t = [hp.tile([128, LOC], bf16, tag=f"hT_{s}", name=f"hT_{s}")
                    for s in range(K1)]
            for s in range(K1):
                nc.sync.dma_start(hT_t[s][:], hT_d[s * 128:(s + 1) * 128, :])
            with tc.tile_pool(name="ppB", bufs=2, space="PSUM") as ppB:
                for nb in range(NCHUNK):
                    ps = ppB.tile([128, 512], f32, tag="feat")
                    psel = ppB.tile([128, 8], f32, tag="fel")
                    pser = ppB.tile([128, 8], f32, tag="fer")
                    blk = slice(nb * 128, (nb + 1) * 128)
                    for s in range(K1):
                        st_ = (s == 0)
                        sp_ = (s == K1 - 1)
                        nc.tensor.matmul(ps[:], hT_t[s][:, blk], W1_t[s],
                                         start=st_, stop=sp_)
                        nc.tensor.matmul(psel[:], hT_t[s][:, blk], Wl1_t[s],
                                         start=st_, stop=sp_)
                        nc.tensor.matmul(pser[:], hT_t[s][:, blk], Wr1_t[s],
                                         start=st_, stop=sp_)
                    fx = wp.tile([128, 768], u8, tag="fx")
                    nc.vector.tensor_copy(fx[:, 0:480].bitcast(bf16), ps[:, 0:240])
                    nc.vector.tensor_copy(fx[:, 480:752].bitcast(f8),
                                          ps[:, 240:512])
                    nc.vector.tensor_copy(fx[:, 752:768].bitcast(bf16), psel[:])
                    nc.sync.dma_start(fx1_loc[blk, :], fx[:])
                    er = cp.tile([128, 8], bf16, tag=f"er1c{nb}", name=f"er1c{nb}")
                    nc.vector.tensor_copy(er[:], pser[:])
                    er1_sb.append(er)
                nc.gpsimd.collective_compute(
                    "AllGather", mybir.AluOpType.bypass,
                    replica_groups=[list(range(NCORES))],
                    ins=[fx1_loc[:]], outs=[fx1_full[:]])
            hw_pool.__exit__(None, None, None)

            # ---- erp precompute: runs during the AllGather (no fx dep) ----
            # stOH is identical for both layers: load it resident once here
            # (43.5 KB/partition) instead of streaming it in both edge loops.
            erp_sb, stA_sb = [], []
            with (
                tc.tile_pool(name="ppPre", bufs=2, space="PSUM") as ppre,
                tc.tile_pool(name="pre", bufs=2) as prp,
            ):
                for ch in range(NCHUNK):
                    st_r = cp.tile([128, nslab * 128], f8, tag=f"stsb{ch}",
                                   name=f"stsb{ch}")
                    nc.sync.dma_start(
                        st_r[:], stOH_d[:, ch * NI:(ch + 1) * NI])
                    stA_sb.append(st_r)
                    sdt = prp.tile([128, nslab * 128], f8, tag="sdt")
                    nc.sync.dma_start(
                        sdt[:], dOffB_d[:, ch * NI:(ch + 1) * NI])
                    erp = ppre.tile([128, nslab * 8], f32, tag="erp")
                    for jj in range(nslab):
                        nc.tensor.matmul(erp[:, jj * 8:(jj + 1) * 8],
                                         sdt[:, jj * 128:(jj + 1) * 128],
                                         er1_sb[ch][:],
                                         start=(jj == 0),
                                         stop=(jj == nslab - 1),
                                         skip_group_check=True)
                    es = cp.tile([128, nslab * 8], f32, tag=f"erpsb{ch}",
                                 name=f"erpsb{ch}")
                    nc.vector.tensor_copy(es[:], erp[:])
                    erp_sb.append(es)

            # ---- phase C: layer-1 edge processing (+ fused layer-2 dense) ----
            with (
                tc.tile_pool(name="ppC", bufs=2, space="PSUM") as ppC,
                tc.tile_pool(name="ppT", bufs=1, space="PSUM") as ppT,
                tc.tile_pool(name="ppD", bufs=1, space="PSUM") as ppD,
                tc.tile_pool(name="edge", bufs=2) as ep,
                tc.tile_pool(name="gbuf", bufs=3) as gp,
                tc.tile_pool(name="xtb", bufs=1) as xtp,
                tc.tile_pool(name="elu", bufs=1) as lp,
            ):
                for ch in range(NCHUNK):
                    stA = stA_sb[ch]
                    erp = erp_sb[ch]
                    gq = []
                    for q in range(NQ):
                        g = gp.tile([128, nqs[q], 768], u8, tag=f"g{q}")
                        nc.gpsimd.dma_gather(
                            g[:], fx1_full[:],
                            srcI_t[:, (ch * nslab + offs[q]) * 8:
                                   (ch * nslab + offs[q + 1]) * 8],
                            num_idxs=nqs[q] * 128, num_idxs_reg=nqs[q] * 128,
                            elem_size=768, single_packet=False,
                            queue_num=q)
                        gq.append(g)
                    pa = ppC.tile([128, 512], f32, tag="agg")
                    pss = ppC.tile([128, 8], f32, tag="ss")
                    for q in range(NQ):
                        g = gq[q]
                        ns0 = offs[q]
                        nb_ = nqs[q]
                        lr = ep.tile([128, nb_, 8], f32, tag=f"lr{q}")
                        nc.vector.tensor_tensor(
                            lr[:], g[:, :, 752:768].bitcast(bf16),
                            erp[:, ns0 * 8:(ns0 + nb_) * 8]
                                .rearrange("p (b n) -> p b n", n=8),
                            Alu.add)
                        nc.vector.scalar_tensor_tensor(
                            lr[:], lr[:], NEG, lr[:], Alu.mult, Alu.max)
                        exb = ep.tile([128, nb_, 8], bf16, tag=f"exb{q}")
                        nc.scalar.activation(exb[:], lr[:], Act.Exp)
                        # alpha-weighting: d-major features, heads innermost
                        # -> packed 2x DVE mode
                        xt = xtp.tile([128, nb_, 512], bf16, tag=f"xt{q}")
                        nc.vector.tensor_tensor(
                            xt[:, :, 0:240]
                                .rearrange("p b (d h) -> p b d h", h=H0),
                            g[:, :, 0:480].bitcast(bf16)
                                .rearrange("p b (d h) -> p b d h", h=H0),
                            exb[:].unsqueeze(2)
                                .broadcast_to([128, nb_, 30, H0]),
                            Alu.mult)
                        nc.vector.tensor_tensor(
                            xt[:, :, 240:512]
                                .rearrange("p b (d h) -> p b d h", h=H0),
                            g[:, :, 480:752].bitcast(f8)
                                .rearrange("p b (d h) -> p b d h", h=H0),
                            exb[:].unsqueeze(2)
                                .broadcast_to([128, nb_, 34, H0]),
                            Alu.mult)
                        for j in range(nb_):
                            jj = ns0 + j
                            stj = stA[:, jj * 128:(jj + 1) * 128]
                            nc.tensor.matmul(pa[:], stj, xt[:, j, :],
                                             start=(jj == 0),
                                             stop=(jj == nslab - 1))
                            nc.tensor.matmul(pss[:], stj, exb[:, j, :],
                                             start=(jj == 0),
                                             stop=(jj == nslab - 1))
                    sden = lp.tile([128, 8], f32, tag="sden")
                    nc.vector.tensor_scalar(sden[:], pss[:], 1e-30, None, Alu.max)
                    rec = lp.tile([128, 8], f32, tag="rec")
                    nc.vector.reciprocal(rec[:], sden[:])
                    x1 = lp.tile([128, 512], f32, tag="x1")
                    nc.vector.tensor_tensor(
                        x1[:].rearrange("p (d h) -> p d h", h=H0),
                        pa[:].rearrange("p (d h) -> p d h", h=H0),
                        rec[:].unsqueeze(1).broadcast_to([128, HID, H0]),
                        Alu.mult)
                    # ELU: relu(v) - 1 + exp(min(v, 0))
                    tmin = lp.tile([128, 512], f32, tag="tmin")
                    nc.vector.tensor_scalar(tmin[:], x1[:], 0.0, None, Alu.min)
                    texp = lp.tile([128, 512], f32, tag="texp")
                    nc.scalar.activation(texp[:], tmin[:], Act.Exp)
                    trelu = lp.tile([128, 512], f32, tag="trelu")
                    nc.scalar.activation(trelu[:], x1[:], Act.Relu)
                    xe = lp.tile([128, 512], f32, tag="xe")
                    nc.vector.scalar_tensor_tensor(xe[:], texp[:], -1.0,
                                                   trelu[:], Alu.add, Alu.add)
                    for s in range(K1):
                        tp = ppT.tile([128, 128], f32, tag="tp")
                        nc.tensor.transpose(tp[:], xe[:, s * 128:(s + 1) * 128],
                                            id_t[:])
                        nc.vector.tensor_copy(
                            xT_t[s][:, ch * 128:(ch + 1) * 128], tp[:])
                    # fused layer-2 dense for this chunk: [W2 | Wl2 | Wr2]
                    # concatenated -> single accumulation group in one bank
                    ps2 = ppD.tile([128, 66], f32, tag="feat2")
                    blk = slice(ch * 128, (ch + 1) * 128)
                    for s in range(K1):
                        nc.tensor.matmul(ps2[:], xT_t[s][:, blk], W2c_t[s],
                                         start=(s == 0), stop=(s == K1 - 1))
                    fx2 = wp.tile([128, 128], u16, tag="fx2")
                    nc.vector.tensor_copy(fx2[:, 0:64].bitcast(bf16), ps2[:, 0:64])
                    nc.vector.tensor_copy(fx2[:, 64:66].bitcast(f32), ps2[:, 64:65])
                    nc.sync.dma_start(fx2_loc[blk, :], fx2[:])
                    er2 = cp.tile([128, 1], bf16, tag=f"er2c{ch}", name=f"er2c{ch}")
                    nc.vector.tensor_copy(er2[:], ps2[:, 65:66])
                    er2_sb.append(er2)
                nc.gpsimd.collective_compute(
                    "AllGather", mybir.AluOpType.bypass,
                    replica_groups=[list(range(NCORES))],
                    ins=[fx2_loc[:]], outs=[fx2_full[:]])

            # ---- erp2 precompute: runs during the second AllGather ----
            erp2_sb = []
            with (
                tc.tile_pool(name="ppPre2", bufs=2, space="PSUM") as ppre2,
                tc.tile_pool(name="pre2", bufs=2) as prp2,
            ):
                for ch in range(NCHUNK):
                    sdt2 = prp2.tile([128, nslab * 128], f8, tag="sdt2")
                    nc.sync.dma_start(
                        sdt2[:], dOffB_d[:, ch * NI:(ch + 1) * NI])
                    erp2 = ppre2.tile([128, nslab], f32, tag="erp2")
                    for jj in range(nslab):
                        nc.tensor.matmul(erp2[:, jj:jj + 1],
                                         sdt2[:, jj * 128:(jj + 1) * 128],
                                         er2_sb[ch][:],
                                         start=(jj == 0),
                                         stop=(jj == nslab - 1),
                                         skip_group_check=True)
                    es2 = cp.tile([128, nslab], f32, tag=f"erp2sb{ch}",
                                  name=f"erp2sb{ch}")
                    nc.vector.tensor_copy(es2[:], erp2[:])
                    erp2_sb.append(es2)

            # ---- phase E: layer-2 edge processing ----
            with (
                tc.tile_pool(name="ppE", bufs=2, space="PSUM") as ppE,
                tc.tile_pool(name="edge2", bufs=2) as e2,
                tc.tile_pool(name="gbuf2", bufs=3) as gp2,
            ):
                for ch in range(NCHUNK):
                    stA2 = stA_sb[ch]
                    erp2 = erp2_sb[ch]
                    gq2 = []
                    for q in range(NQ):
                        g2 = gp2.tile([128, nqs[q], 128], u16, tag=f"g2{q}")
                        nc.gpsimd.dma_gather(
                            g2[:], fx2_full[:],
                            srcI_t[:, (ch * nslab + offs[q]) * 8:
                                   (ch * nslab + offs[q + 1]) * 8],
                            num_idxs=nqs[q] * 128, num_idxs_reg=nqs[q] * 128,
                            elem_size=128, single_packet=False,
                            queue_num=q)
                        gq2.append(g2)
                    pa2 = ppE.tile([128, 65], f32, tag="agg2")
                    for q in range(NQ):
                        g2 = gq2[q]
                        ns0 = offs[q]
                        nb_ = nqs[q]
                        lr2 = e2.tile([128, nb_, 1], f32, tag=f"lr2{q}")
                        nc.vector.tensor_tensor(
                            lr2[:], g2[:, :, 64:66].bitcast(f32),
                            erp2[:, ns0:ns0 + nb_]
                                .rearrange("p (b n) -> p b n", n=1),
                            Alu.add)
                        nc.vector.scalar_tensor_tensor(
                            lr2[:], lr2[:], NEG, lr2[:], Alu.mult, Alu.max)
                        exb2 = e2.tile([128, nb_, 1], bf16, tag=f"exb2{q}")
                        nc.scalar.activation(exb2[:], lr2[:], Act.Exp)
                        xt2 = e2.tile([128, nb_, 65], bf16, tag=f"xt2{q}")
                        nc.vector.tensor_tensor(
                            xt2[:, :, 0:64], g2[:, :, 0:64].bitcast(bf16),
                            exb2[:].broadcast_to([128, nb_, 64]),
                            Alu.mult)
                        nc.vector.tensor_copy(xt2[:, :, 64:65], exb2[:])
                        for j in range(nb_):
                            jj = ns0 + j
                            stj = stA2[:, jj * 128:(jj + 1) * 128]
                            nc.tensor.matmul(pa2[:], stj, xt2[:, j, :],
                                             start=(jj == 0),
                                             stop=(jj == nslab - 1))
                    sden2 = e2.tile([128, 1], f32, tag="sden2")
                    nc.vector.tensor_scalar(sden2[:], pa2[:, 64:65], 1e-30, None,
                                            Alu.max)
                    rec2 = e2.tile([128, 1], f32, tag="rec2")
                    nc.vector.reciprocal(rec2[:], sden2[:])
                    outf = e2.tile([128, 64], f32, tag="outf")
                    nc.vector.tensor_scalar(outf[:], pa2[:, 0:64], rec2[:, 0:1],
                                            None, Alu.mult)
                    nc.sync.dma_start(y_d[ch * 128:(ch + 1) * 128, :], outf[:])

    nc.compile()
    return nc


def _wrap_idx(a):
    """flat int array -> [128, n//16] int16 dma_gather index layout."""
    w = a.reshape(-1, 16).T.astype(np.int16)
    return np.tile(w, (8, 1))


def _prep_inputs(h, src, dst, W1, attn_l1, attn_r1, W2, attn_l2, attn_r2):
    src = np.asarray(src)
    dst = np.asarray(dst)
    h = np.asarray(h, dtype=np.float32)
    W1 = np.asarray(W1, dtype=np.float32)
    W2 = np.asarray(W2, dtype=np.float32)
    attn_l1 = np.asarray(attn_l1, np.float32)
    attn_r1 = np.asarray(attn_r1, np.float32)
    attn_l2 = np.asarray(attn_l2, np.float32)
    attn_r2 = np.asarray(attn_r2, np.float32)

    core_of = dst // NPER
    dloc_all = dst % NPER

    per_core = []
    nslab = 1
    for c in range(NCORES):
        ids = np.nonzero(core_of == c)[0]
        order = np.argsort(dloc_all[ids], kind="stable")
        ids = ids[order]
        dl = dloc_all[ids]
        ch = dl // 128
        cnt = np.bincount(ch, minlength=NCHUNK)
        nslab = max(nslab, int(np.ceil(cnt.max() / 128)))
        per_core.append((ids, dl, cnt))

    EPAD = NCHUNK * nslab * 128
    src_pad_row = 0   # pad edges: any valid row; one-hot col 200 zeroes them

    # padded-global row index for every node
    pg = (np.arange(N_NODES) // NPER) * LOC + (np.arange(N_NODES) % NPER)

    # d-major permutation of layer-1 features: f' = d*H0 + h  <-  f = h*HID + d
    # permd[f'] = original column index
    dd, hh = np.meshgrid(np.arange(HID), np.arange(H0), indexing="ij")
    permd = (hh * HID + dd).reshape(-1)   # [512]

    W1p = W1[:, permd]                     # columns d-major
    Wl1 = W1 @ _expand_attn(attn_l1)       # [512, 8]
    Wr1 = W1 @ _expand_attn(attn_r1)
    # layer-2 input features are d-major -> permute W2/Wl2/Wr2 rows;
    # concatenate [W2 | Wl2 | Wr2] for a single accumulation group
    W2p = W2[permd, :]
    Wl2 = (W2 @ attn_l2.reshape(1, OUT_D).T)[permd]   # [512, 1]
    Wr2 = (W2 @ attn_r2.reshape(1, OUT_D).T)[permd]
    W2c = np.concatenate([W2p, Wl2, Wr2], axis=1)     # [512, 66]
    ident = np.eye(128, dtype=np.float32)

    in_maps = []
    for c in range(NCORES):
        ids, dl, cnt = per_core[c]
        src_g = np.full(EPAD, src_pad_row, np.int64)
        doff = np.full(EPAD, 200, np.int64)  # pad edges miss the one-hot
        pos = 0
        for k in range(NCHUNK):
            sel = ids[pos:pos + cnt[k]]
            dsel = dl[pos:pos + cnt[k]]
            base = k * nslab * 128
            src_g[base:base + cnt[k]] = pg[src[sel]]
            doff[base:base + cnt[k]] = dsel - 128 * k
            pos += cnt[k]
        rng128 = np.arange(128)
        doffb_oh = (doff[None, :] == rng128[:, None]).astype(ml_dtypes.float8_e4m3)
        D = doff.reshape(-1, 128)
        st_oh = np.ascontiguousarray(
            (D[:, :, None] == rng128[None, None, :])
            .transpose(1, 0, 2).reshape(128, EPAD)).astype(ml_dtypes.float8_e4m3)
        hc = np.zeros((IN_DIM, LOC), ml_dtypes.bfloat16)
        hc[:, :NPER] = h[c * NPER:(c + 1) * NPER].T.astype(ml_dtypes.bfloat16)
        in_maps.append({
            "hT": hc,
            "W1": W1p.astype(ml_dtypes.bfloat16),
            "Wl1": Wl1.astype(ml_dtypes.bfloat16),
            "Wr1": Wr1.astype(ml_dtypes.bfloat16),
            "W2c": W2c.astype(ml_dtypes.bfloat16),
            "srcI": _wrap_idx(src_g),
            "dOffB": doffb_oh, "stOH": st_oh,
            "ident": ident,
        })
    return nslab, in_maps


def _expand_attn(attn):
    """attn [H0, HID] -> [H0*HID, H0] block-diag so el = feat @ out."""
    A = np.zeros((IN_DIM, H0), np.float32)
    for hh in range(H0):
        A[hh * HID:(hh + 1) * HID, hh] = attn[hh]
    return A


def kernel(h, src, dst, W1, attn_l1, attn_r1, W2, attn_l2, attn_r2,
           _trace=False):
    from concourse.bass_utils import run_bass_kernel_spmd

    nslab, in_maps = _prep_inputs(h, src, dst, W1, attn_l1, attn_r1,
                                  W2, attn_l2, attn_r2)
    if nslab not in _cache:
        _cache[nslab] = _build(nslab)
    nc = _cache[nslab]

    if _trace:
        _install_trace_hook()
    res = run_bass_kernel_spmd(nc, in_maps, list(range(NCORES)), trace=_trace)
    out = np.concatenate([res.results[c]["y"][:NPER] for c in range(NCORES)], axis=0)
    if _trace:
        return out, res
    return out


def _install_trace_hook():
    import sys, types
    if "antenv.axon_hooks" in sys.modules:
        return
    try:
        import antenv
        from trn_agent_boot.trn_boot import _ntff_profile_via_ctypes
    except ImportError:
        return
    mod = types.ModuleType("antenv.axon_hooks")
    state = {"hook": None}
    mod.set_axon_ntff_profile_hook = lambda hk: state.__setitem__("hook", hk)
    mod.get_axon_ntff_profile_hook = lambda: state["hook"]
    sys.modules["antenv.axon_hooks"] = mod
    antenv.axon_hooks = mod
    try:
        mod.set_axon_ntff_profile_hook(
            _ntff_profile_via_ctypes("/opt/axon/libaxon_pjrt.so"))
    except Exception:
        pass
